# revision 1
# baseline (speedup 1.0000x reference)
"""Trainium2 Bass kernel for nn_NodeNet (GNN message passing + 15-qubit circuit).

Algebraic structure exploited (all exact):
1. The joint 2^15 state stays a tensor product of small components — gates
   only entangle qubits incrementally and only <Z_5>, <Z_10> are measured.
   We evolve per-component states, merging (outer product) only when a CNOT
   first crosses two components.
2. An RY on a still-unentangled qubit is an angle addition on its (cos,sin)
   pair, so every R preceding the qubit's first CNOT folds into angle prep;
   same-bit RYs separated only by commuting gates also merge (angle sum).
3. Adjacent identical CNOT pairs with nothing touching either qubit in
   between cancel (C^2 = I). Afterwards qubits 8, 9, 12 never entangle with
   a measured qubit and drop out; the q10 component ends 16-dim, q5 256-dim.

Layout: 128 graph nodes = 128 SBUF partitions; per-node angles are
per-partition scalars; gates are strided free-dim vector ops (merges are
single tensor_tensor ops on double-broadcast outer-product views); the
message-passing matmuls run on the PE.

Self-contained: hardcodes shapes (N=128, E=1024) and the gate list.
"""

import math

import numpy as np

# (op, arg1, arg2): ('R', theta_index, wire) or ('C', control, target)
GATES = [
    ('R', 0, 0), ('R', 1, 1), ('C', 0, 1),
    ('R', 2, 2), ('R', 3, 3), ('C', 3, 2),
    ('R', 4, 4), ('R', 5, 5), ('C', 4, 5),
    ('R', 6, 6), ('R', 7, 7), ('C', 7, 6),
    ('R', 8, 8), ('R', 9, 9), ('C', 8, 9),
    ('R', 10, 10), ('R', 11, 11), ('C', 11, 10),
    ('R', 12, 12), ('R', 13, 13), ('C', 8, 9),
    ('R', 14, 14),
    ('R', 15, 1), ('R', 16, 2), ('C', 1, 2),
    ('R', 14, 5), ('R', 15, 6), ('C', 6, 5),
    ('R', 16, 9), ('R', 17, 10), ('C', 9, 10),
    ('R', 18, 13), ('R', 19, 14), ('C', 9, 10),
    ('R', 19, 2), ('R', 20, 5), ('C', 2, 5),
    ('R', 21, 10), ('R', 22, 13), ('C', 13, 10),
    ('R', 23, 5), ('R', 24, 10),
    ('R', 25, 0), ('R', 26, 5), ('C', 0, 5),
    ('R', 27, 10), ('R', 28, 14), ('C', 14, 10),
    ('R', 29, 5), ('R', 30, 10),
]

N_QUBITS = 15
MEASURED = (5, 10)
N_CORES = 8
PI = math.pi


def build_plan():
    """Returns (folds, ops, measure, dims).

    folds: [(qubit, theta_idx)] — R gates folded into initial angles.
    ops: sequence of
      ('RY', cid, bit, (theta_idx, ...))   # angle = sum of thetas
      ('CNOT', cid, control_bit, target_bit)
      ('MERGE', ca, cb, new_cid, da, db)   # ca = low bits, cb = high bits
    measure: {qubit: (cid, bit)}.
    dims: {cid: dim} (single-qubit cid is ('q', q)).
    """
    # Pass 1: cancel adjacent identical CNOT pairs (C^2 = I) with no
    # intervening gate touching either qubit. Iterate to fixpoint.
    gates = list(GATES)
    changed = True
    while changed:
        changed = False
        for i, g in enumerate(gates):
            if g[0] != 'C':
                continue
            for j in range(i + 1, len(gates)):
                h = gates[j]
                touched = {h[2]} if h[0] == 'R' else {h[1], h[2]}
                if h == g:
                    del gates[j]
                    del gates[i]
                    changed = True
                    break
                if touched & {g[1], g[2]}:
                    break
            if changed:
                break

    # Pass 2: fold + merge-on-demand.
    comp_of = {q: ('q', q) for q in range(N_QUBITS)}
    bits = {('q', q): {q: 0} for q in range(N_QUBITS)}
    dims = {('q', q): 2 for q in range(N_QUBITS)}
    folds = []
    ops = []
    nid_counter = [0]

    for op, a, b in gates:
        if op == 'R':
            q = b
            c = comp_of[q]
            if dims[c] == 2:
                folds.append((q, a))
            else:
                ops.append(['RY', c, bits[c][q], [a]])
        else:
            ca, cb = comp_of[a], comp_of[b]
            if ca != cb:
                nid = ('m', nid_counter[0])
                nid_counter[0] += 1
                da, db = dims[ca], dims[cb]
                shift = int(math.log2(da))
                nb = dict(bits[ca])
                for q2, bit in bits[cb].items():
                    nb[q2] = bit + shift
                bits[nid] = nb
                dims[nid] = da * db
                ops.append(['MERGE', ca, cb, nid, da, db])
                for q2 in nb:
                    comp_of[q2] = nid
            c = comp_of[a]
            ops.append(['CNOT', c, bits[c][a], bits[c][b]])

    # Pass 3: merge same-bit RYs separated only by commuting ops on the
    # same component (ops on other components always commute).
    changed = True
    while changed:
        changed = False
        for i, o in enumerate(ops):
            if o[0] != 'RY':
                continue
            cid, b = o[1], o[2]
            for j in range(i - 1, -1, -1):
                p = ops[j]
                pc = p[3] if p[0] == 'MERGE' else p[1]
                if pc != cid:
                    continue  # different component: commutes
                if p[0] == 'RY' and p[2] == b:
                    p[3] = list(p[3]) + list(o[3])
                    del ops[i]
                    changed = True
                elif p[0] == 'RY':
                    continue  # RYs on different bits commute
                break
            if changed:
                break

    # Pass 3b: sink RYs through merges into subcomponents (an RY commutes
    # with everything not touching its bit; through a MERGE it retargets
    # the sub-component with the bit remapped).
    changed = True
    while changed:
        changed = False
        for i, o in enumerate(ops):
            if o[0] != 'RY':
                continue
            cid, b = o[1], o[2]
            j = i - 1
            while j >= 0:
                p = ops[j]
                if p[0] == 'MERGE' and p[3] == cid:
                    da = p[4]
                    shift = int(math.log2(da))
                    if b < shift:
                        o[1], sub = p[1], p[1]
                    else:
                        o[1], sub = p[2], p[2]
                        o[2] = b - shift
                    ops.insert(j, ops.pop(i))
                    changed = True
                    break
                pc = p[3] if p[0] == 'MERGE' else p[1]
                if pc != cid:
                    j -= 1
                    continue
                if p[0] == 'RY':
                    j -= 1
                    continue
                if p[0] == 'CNOT' and b not in (p[2], p[3]):
                    j -= 1
                    continue
                break
            if changed:
                break
    # re-run same-bit RY merging after sinking
    changed = True
    while changed:
        changed = False
        for i, o in enumerate(ops):
            if o[0] != 'RY':
                continue
            cid, b = o[1], o[2]
            for j in range(i - 1, -1, -1):
                p = ops[j]
                pc = p[3] if p[0] == 'MERGE' else p[1]
                if pc != cid:
                    continue
                if p[0] == 'RY' and p[2] == b:
                    p[3] = list(p[3]) + list(o[3])
                    del ops[i]
                    changed = True
                elif p[0] == 'RY':
                    continue
                break
            if changed:
                break

    # Pass 4: prune components that never reach a measured qubit.
    needed = {comp_of[q] for q in MEASURED}
    changed = True
    while changed:
        changed = False
        for o in ops:
            if o[0] == 'MERGE' and o[3] in needed:
                if o[1] not in needed or o[2] not in needed:
                    needed.add(o[1])
                    needed.add(o[2])
                    changed = True
    ops = [o for o in ops
           if (o[0] == 'MERGE' and o[3] in needed)
           or (o[0] != 'MERGE' and o[1] in needed)]
    needed_qubits = {q for cid in needed if cid[0] == 'm'
                     for q in bits[cid]} | set(MEASURED)
    folds = [(q, a) for q, a in folds if q in needed_qubits]

    # Pass 5: fuse a final RY on the measured bit into the measurement:
    # z' = cos(th)*(2*sum(a0^2) - 1) - 2*sin(th)*sum(a0*a1).
    measure = {}
    for q in MEASURED:
        c = comp_of[q]
        mb = bits[c][q]
        fuse = None
        for i in range(len(ops) - 1, -1, -1):
            o = ops[i]
            oc = o[3] if o[0] == 'MERGE' else o[1]
            if oc != c:
                continue
            if o[0] == 'RY' and o[2] == mb:
                fuse = tuple(o[3])
                del ops[i]
            break
        measure[q] = (c, mb, fuse)

    ops = [tuple(o[:3]) + (tuple(o[3]),) if o[0] == 'RY' else tuple(o)
           for o in ops]
    return folds, ops, measure, dims


FOLDS, OPS, MEASURE, DIMS = build_plan()

# Angle-column layout: cols 0..14 = M(+folded thetas); 15..45 = theta;
# extra columns for multi-theta RY sums and fused-measure double angles.
TH0 = 15
_extra = []
RY_COL = {}


def _alloc_col(ths):
    if len(ths) == 1:
        RY_COL[ths] = TH0 + ths[0]
    elif ths not in RY_COL:
        RY_COL[ths] = TH0 + 31 + len(_extra)
        _extra.append(ths)


for _o in OPS:
    if _o[0] == 'RY':
        _alloc_col(_o[3])
for _q, (_c, _b, _fuse) in MEASURE.items():
    if _fuse is not None:
        _alloc_col(_fuse + _fuse)  # full angle = 2 * half angle
EXTRA_SUMS = list(_extra)
NANG = TH0 + 31 + len(EXTRA_SUMS)

_cache = {}


def _build_program():
    import concourse.bacc as bacc
    import concourse.mybir as mybir
    import concourse.tile as tile
    from concourse.masks import make_identity

    f32 = mybir.dt.float32
    i32 = mybir.dt.int32
    Alu = mybir.AluOpType
    Act = mybir.ActivationFunctionType

    nc = bacc.Bacc(
        "TRN2",
        target_bir_lowering=False,
        debug=False,
        enable_asserts=False,
        num_devices=1,
    )

    X_d = nc.dram_tensor("X", [128, 5], f32, kind="ExternalInput").ap()
    e_d = nc.dram_tensor("e", [1024], f32, kind="ExternalInput").ap()
    Ri_d = nc.dram_tensor("Ri", [128, 1024], f32, kind="ExternalInput").ap()
    Ro_d = nc.dram_tensor("Ro", [128, 1024], f32, kind="ExternalInput").ap()
    th_d = nc.dram_tensor("theta", [31], f32, kind="ExternalInput").ap()
    out_d = nc.dram_tensor("out", [128, 2], f32, kind="ExternalOutput").ap()

    with tile.TileContext(nc) as tc:
        with (
            tc.tile_pool(name="sbuf", bufs=1) as sb,
            tc.tile_pool(name="psum", bufs=1, space="PSUM") as ps,
            tc.tile_pool(name="pstp", bufs=2, space="PSUM") as pstp,
            tc.tile_pool(name="psmm", bufs=2, space="PSUM") as psmm,
        ):
            # ---------- inputs ----------
            X_sb = sb.tile([128, 5], f32, tag="X")
            Ri_sb = sb.tile([128, 1024], f32, tag="Ri")
            Ro_sb = sb.tile([128, 1024], f32, tag="Ro")
            e8_sb = sb.tile([8, 128], f32, tag="e8")
            th_sb = sb.tile([1, 31], f32, tag="th")
            nc.sync.dma_start(th_sb[:], th_d.unsqueeze(0))
            nc.sync.dma_start(e8_sb[:], e_d.rearrange("(c p) -> c p", c=8))
            nc.sync.dma_start(X_sb[:], X_d)
            for h in range(2):
                nc.sync.dma_start(Ri_sb[:, h * 512:(h + 1) * 512],
                                  Ri_d[:, h * 512:(h + 1) * 512])
                nc.sync.dma_start(Ro_sb[:, h * 512:(h + 1) * 512],
                                  Ro_d[:, h * 512:(h + 1) * 512])

            # ---------- constants ----------
            ident = sb.tile([128, 128], f32, tag="ident")
            make_identity(nc, ident[:])
            pio2 = sb.tile([128, 1], f32, tag="pio2")
            nc.gpsimd.memset(pio2[:], PI / 2.0)
            ones1 = sb.tile([1, 128], f32, tag="ones1")
            nc.gpsimd.memset(ones1[:], 1.0)
            rrC = sb.tile([128, 1], f32, tag="rrC")
            nc.gpsimd.memset(rrC[:], 16.0)

            # preload ACT function tables during the DMA window
            warm = sb.tile([1, 1], f32, tag="warm")
            nc.gpsimd.memset(warm[:], 0.0)
            nc.scalar.activation(warm[:], warm[:], Act.Sin)
            nc.scalar.activation(warm[:], warm[:], Act.Abs)
            nc.scalar.activation(warm[:], warm[:], Act.Identity,
                                 bias=rrC[0:1, :], scale=0.0)
            nc.scalar.copy(warm[:], warm[:])

            # ---------- ANG: [M + folds | theta | extra sums] ----------
            ANG = sb.tile([128, NANG], f32, tag="ANG")

            # broadcast theta into ANG[:, 15:46] via K=1 matmul
            th_ps = psmm.tile([128, 31], f32, tag="mm")
            nc.tensor.matmul(th_ps[:], ones1[:], th_sb[:], start=True, stop=True)
            nc.scalar.copy(ANG[:, TH0:TH0 + 31], th_ps[:])

            # e into per-edge-partition layout [128, 8]
            e_ps = psmm.tile([128, 8], f32, tag="mm")
            nc.tensor.transpose(e_ps[:], e8_sb[:], ident[0:8, 0:8])
            e_sb = sb.tile([128, 8], f32, tag="e_sb")
            nc.scalar.copy(e_sb[:], e_ps[:])

            # ---------- bo/bi chunks + weight by e (batched) ----------
            # psum [128, 40] holds (bo_c | bi_c) for 4 chunks; weight with a
            # broadcast-e tensor_tensor into SBUF.
            bowbiw = sb.tile([128, 80], f32, tag="bowbiw")  # 8 chunks x 10
            for h in range(2):
                bb_ps = psmm.tile([128, 40], f32, tag="bb")
                for cc in range(4):
                    c = h * 4 + cc
                    nc.tensor.matmul(
                        bb_ps[:, cc * 10:cc * 10 + 5],
                        Ro_sb[:, c * 128:(c + 1) * 128], X_sb[:],
                        start=True, stop=True,
                    )
                    nc.tensor.matmul(
                        bb_ps[:, cc * 10 + 5:cc * 10 + 10],
                        Ri_sb[:, c * 128:(c + 1) * 128], X_sb[:],
                        start=True, stop=True,
                    )
                ev = e_sb[:, h * 4:(h + 1) * 4].rearrange(
                    "p (c o) -> p c o", o=1).to_broadcast((128, 4, 10))
                ov = bowbiw[:, h * 40:(h + 1) * 40].rearrange(
                    "p (c j) -> p c j", j=10)
                iv = bb_ps[:].rearrange("p (c j) -> p c j", j=10)
                nc.vector.tensor_tensor(ov, iv, ev, Alu.mult)

            # ---------- transpose Ri, Ro chunks; 4 per PSUM bank ----------
            RiT = sb.tile([128, 1024], f32, tag="RiT")
            RoT = sb.tile([128, 1024], f32, tag="RoT")
            for h in range(2):
                for mat, matT, eng in ((Ri_sb, RiT, "act"), (Ro_sb, RoT, "dve")):
                    tp = pstp.tile([128, 512], f32, tag="tp")
                    for cc in range(4):
                        c = h * 4 + cc
                        nc.tensor.transpose(
                            tp[:, cc * 128:(cc + 1) * 128],
                            mat[:, c * 128:(c + 1) * 128], ident[:])
                    dst = matT[:, h * 512:(h + 1) * 512]
                    if eng == "act":
                        nc.scalar.copy(dst, tp[:])
                    else:
                        nc.vector.tensor_copy(dst, tp[:])

            # ---------- mi = (Ri*e) @ bo, mo = (Ro*e) @ bi ----------
            mi_ps = ps.tile([128, 5], f32, tag="mi")
            mo_ps = ps.tile([128, 5], f32, tag="mo")
            for c in range(8):
                nc.tensor.matmul(
                    mi_ps[:], RiT[:, c * 128:(c + 1) * 128],
                    bowbiw[:, c * 10:c * 10 + 5],
                    start=(c == 0), stop=(c == 7),
                )
                nc.tensor.matmul(
                    mo_ps[:], RoT[:, c * 128:(c + 1) * 128],
                    bowbiw[:, c * 10 + 5:c * 10 + 10],
                    start=(c == 0), stop=(c == 7),
                )
            nc.scalar.copy(ANG[:, 0:5], mi_ps[:])
            nc.scalar.copy(ANG[:, 5:10], mo_ps[:])
            nc.scalar.copy(ANG[:, 10:15], X_sb[:])

            # ---------- folds (batched runs) + extra sums ----------
            def batch_runs(pairs):
                """pairs: [(dst_col, src_col)]; emit TT adds on maximal runs
                with dst stride 1 and uniform src stride."""
                pairs = sorted(pairs)
                i = 0
                while i < len(pairs):
                    j = i + 1
                    if j < len(pairs):
                        sd = pairs[j][1] - pairs[i][1]
                        while (j < len(pairs)
                               and pairs[j][0] == pairs[j - 1][0] + 1
                               and pairs[j][1] == pairs[j - 1][1] + sd):
                            j += 1
                    n = j - i
                    d0, s0 = pairs[i]
                    dst = ANG[:, d0:d0 + n]
                    if n == 1:
                        src = ANG[:, s0:s0 + 1]
                    else:
                        src = ANG[:].rearrange(
                            "p (o x) -> p o x", o=1)[:, :, s0:s0 + (n - 1) * sd + 1:sd]
                        dst = ANG[:].rearrange(
                            "p (o x) -> p o x", o=1)[:, :, d0:d0 + n]
                    nc.vector.tensor_tensor(dst, dst, src, Alu.add)
                    i = j

            fold_pairs = [(q, TH0 + a) for q, a in FOLDS]
            # multiple folds to the same qubit must be separate adds
            seen = {}
            rounds = []
            for q, s in fold_pairs:
                k = seen.get(q, 0)
                seen[q] = k + 1
                while len(rounds) <= k:
                    rounds.append([])
                rounds[k].append((q, s))
            for r in rounds:
                batch_runs(r)
            for idx, ths in enumerate(EXTRA_SUMS):
                dcol = TH0 + 31 + idx
                nc.vector.tensor_copy(ANG[:, dcol:dcol + 1],
                                      ANG[:, TH0 + ths[0]:TH0 + ths[0] + 1])
                for a in ths[1:]:
                    nc.vector.tensor_tensor(
                        ANG[:, dcol:dcol + 1], ANG[:, dcol:dcol + 1],
                        ANG[:, TH0 + a:TH0 + a + 1], Alu.add)

            # ---------- range-reduced sin/cos of ANG/2 ----------
            # t = u/(4pi)+C; k = int(t) (trunc or round both fine);
            # f = t-k in [-.5, 1); g = (f > .5); w = f-g in [-.5, .5];
            # sin(u/2) = Sin(2pi*w), cos(u/2) = Sin(pi/2 - 2pi*|w|).
            # Split: theta columns [15:NANG] run early (hidden under the
            # Ri/Ro DMAs + matmuls); M columns [0:15] run once mi/mo land.
            cA = sb.tile([128, NANG], f32, tag="cA")
            sA = sb.tile([128, NANG], f32, tag="sA")
            t_t = sb.tile([128, NANG], f32, tag="rr_t")
            k_i = sb.tile([128, NANG], i32, tag="rr_ki")
            k_f = sb.tile([128, NANG], f32, tag="rr_kf")
            w_t = sb.tile([128, NANG], f32, tag="rr_w")
            g_t = sb.tile([128, NANG], f32, tag="rr_g")

            def sincos(lo, hi):
                sl = slice(lo, hi)
                nc.vector.tensor_scalar(
                    t_t[:, sl], ANG[:, sl], 0.5 / (2.0 * PI), 16.0,
                    Alu.mult, Alu.add)
                nc.vector.tensor_copy(k_i[:, sl], t_t[:, sl])
                nc.vector.tensor_copy(k_f[:, sl], k_i[:, sl])
                nc.vector.tensor_tensor(w_t[:, sl], t_t[:, sl], k_f[:, sl],
                                        Alu.subtract)
                nc.vector.tensor_scalar(g_t[:, sl], w_t[:, sl], 0.5, None,
                                        Alu.is_gt)
                nc.vector.scalar_tensor_tensor(
                    w_t[:, sl], g_t[:, sl], -1.0, w_t[:, sl],
                    Alu.mult, Alu.add)
                nc.scalar.activation(sA[:, sl], w_t[:, sl], Act.Sin,
                                     scale=2.0 * PI)
                nc.scalar.activation(g_t[:, sl], w_t[:, sl], Act.Abs)
                nc.scalar.activation(cA[:, sl], g_t[:, sl], Act.Sin,
                                     bias=pio2[:], scale=-2.0 * PI)

            sincos(TH0, NANG)
            sincos(0, TH0)

            # ---------- single-qubit (cos, sin) pairs: v2[:, 2q:2q+2] -----
            v2 = sb.tile([128, 32], f32, tag="v2")
            v2v = v2[:].rearrange("p (o t) -> p o t", t=2)
            nc.vector.tensor_copy(v2v[:, 0:15, 0], cA[:, 0:15])
            nc.vector.tensor_copy(v2v[:, 0:15, 1], sA[:, 0:15])

            # ---------- component evolution ----------
            tiles = {}
            for q in range(N_QUBITS):
                tiles[('q', q)] = v2[:, 2 * q:2 * q + 2]

            maxdim = max(DIMS.values())
            Dtiles = {}

            def ry(cid, b, ths):
                """3-op RY: D = v*s; a0 = a0*c - D1; a1 = a1*c + D0.
                Per-component D scratch so independent chains don't
                false-share; big D-mults go to ACT, small stay on DVE."""
                v, F = tiles[cid], DIMS[cid]
                if cid not in Dtiles:
                    dt_new = sb.tile([128, F], f32, tag=f"D{cid[1]}")
                    Dtiles[cid] = dt_new
                tmpD = Dtiles[cid]
                col = RY_COL[ths]
                c_ap = cA[:, col:col + 1]
                s_ap = sA[:, col:col + 1]
                view = v.rearrange("p (o t i) -> p o t i", t=2, i=1 << b)
                a0 = view[:, :, 0, :]
                a1 = view[:, :, 1, :]
                D = tmpD[:, 0:F].rearrange("p (o t i) -> p o t i", t=2, i=1 << b)
                D0 = D[:, :, 0, :]
                D1 = D[:, :, 1, :]
                nc.vector.tensor_scalar(tmpD[:, 0:F], v, s_ap, None,
                                        Alu.mult)
                nc.vector.scalar_tensor_tensor(
                    a0, a0, c_ap, D1, Alu.mult, Alu.subtract)
                nc.vector.scalar_tensor_tensor(
                    a1, a1, c_ap, D0, Alu.mult, Alu.add)

            cnt = [0]

            def cnot(cid, bc_, bt):
                """2-copy CNOT into a fresh tile (control=0 half verbatim,
                control=1 half with target slices swapped)."""
                v, F = tiles[cid], DIMS[cid]
                new = sb.tile([128, F], f32, tag=f"cn{cnt[0]}")
                cnt[0] += 1
                hi, lo = max(bc_, bt), min(bc_, bt)
                m = 1 << (hi - lo - 1)
                i = 1 << lo
                ov = new[:].rearrange(
                    "p (o a m b i) -> p o a m b i", a=2, b=2, m=m, i=i)
                iv = v.rearrange(
                    "p (o a m b i) -> p o a m b i", a=2, b=2, m=m, i=i)
                if bc_ == hi:
                    # control = a: copy a=0 plain; a=1 with b reversed
                    nc.vector.tensor_copy(ov[:, :, 0], iv[:, :, 0])
                    nc.vector.tensor_copy(ov[:, :, 1], iv[:, :, 1, :, ::-1])
                else:
                    # control = b: copy b=0 plain; b=1 with a reversed
                    nc.vector.tensor_copy(ov[:, :, :, :, 0], iv[:, :, :, :, 0])
                    nc.vector.tensor_copy(ov[:, :, :, :, 1],
                                          iv[:, :, ::-1][:, :, :, :, 1])
                tiles[cid] = new[:]

            def merge(ca, cb, nid, da, db):
                """1-op outer product via double-broadcast tensor_tensor."""
                L, H = tiles[ca], tiles[cb]
                new = sb.tile([128, da * db], f32, tag=f"c{nid[1]}")
                Lb = L.rearrange("p (o v) -> p o v", o=1).to_broadcast(
                    (128, db, da))
                Hb = H.rearrange("p (w o) -> p w o", o=1).to_broadcast(
                    (128, db, da))
                ov = new[:].rearrange("p (w v) -> p w v", v=da)
                nc.vector.tensor_tensor(ov, Lb, Hb, Alu.mult)
                tiles[nid] = new[:]

            def merge_cnot(ca, cb, nid, da, db, bc_, bt):
                """Fused merge + CNOT in 2 TT ops. Requires control bit in
                the L component (bc_ < log2(da)) and target in H: the
                control=1 half of the outer product reads H with the target
                bit's slices swapped (negative-stride view)."""
                S = int(math.log2(da))
                assert bc_ < S <= bt
                L, H = tiles[ca], tiles[cb]
                new = sb.tile([128, da * db], f32, tag=f"c{nid[1]}")
                v1, v0 = da >> (bc_ + 1), 1 << bc_
                tbh = bt - S
                w1, w0 = db >> (tbh + 1), 1 << tbh
                # out [p, w1, tb, w0, v1, cb, v0]
                ov = new[:].rearrange(
                    "p (w1 tb w0 v1 cb v0) -> p w1 tb w0 v1 cb v0",
                    tb=2, cb=2, w0=w0, v0=v0, w1=w1, v1=v1)
                Lv = L.rearrange("p (v1 cb v0) -> p v1 cb v0", cb=2, v0=v0)
                Hv = H.rearrange("p (w1 tb w0) -> p w1 tb w0", tb=2, w0=w0)
                for cbit in range(2):
                    o_h = ov[:, :, :, :, :, cbit, :]
                    Lh = Lv[:, :, cbit, :].unsqueeze(1).unsqueeze(1).unsqueeze(1)
                    Hh = Hv if cbit == 0 else Hv[:, :, ::-1, :]
                    Hh = Hh.unsqueeze(4).unsqueeze(5)
                    nc.vector.tensor_tensor(
                        o_h.squeeze(),
                        Lh.to_broadcast((128, w1, 2, w0, v1, v0)).squeeze(),
                        Hh.to_broadcast((128, w1, 2, w0, v1, v0)).squeeze(),
                        Alu.mult)
                tiles[nid] = new[:]

            skip_next = [False]
            for oi, o in enumerate(OPS):
                if skip_next[0]:
                    skip_next[0] = False
                    continue
                if o[0] == 'RY':
                    ry(o[1], o[2], o[3])
                elif o[0] == 'CNOT':
                    cnot(o[1], o[2], o[3])
                else:
                    nxt = OPS[oi + 1] if oi + 1 < len(OPS) else None
                    da, S = o[4], int(math.log2(o[4]))
                    if (nxt is not None and nxt[0] == 'CNOT'
                            and nxt[1] == o[3] and nxt[2] < S <= nxt[3]):
                        merge_cnot(o[1], o[2], o[3], o[4], o[5],
                                   nxt[2], nxt[3])
                        skip_next[0] = True
                    else:
                        merge(o[1], o[2], o[3], o[4], o[5])

            # ---------- measurement ----------
            # Plain: z = 2*sum(a0^2) - 1 (unit norm).
            # With a fused final RY(th) on the measured bit:
            #   z = cos(th)*(2*sum(a0^2) - 1) - 2*sin(th)*sum(a0*a1).
            out_sb = sb.tile([128, 2], f32, tag="out")
            zacc = sb.tile([128, 8], f32, tag="zacc")

            for col, q in enumerate(MEASURED):
                cid, b, fuse = MEASURE[q]
                v, F = tiles[cid], DIMS[cid]
                view = v.rearrange("p (o t i) -> p o t i", t=2, i=1 << b)
                a0 = view[:, :, 0, :]
                a1 = view[:, :, 1, :]
                sq = sb.tile([128, F // 2], f32, tag=f"sq{col}")
                sqv = sq[:].rearrange("p (o i) -> p o i", i=1 << b)
                zA = zacc[:, 4 * col:4 * col + 1]
                nc.vector.scalar_tensor_tensor(
                    sqv, a0, 1.0, a0, Alu.mult, Alu.mult, accum_out=zA)
                if fuse is None:
                    # out = pi*(1 - z) = z0*(-2pi) + 2pi
                    nc.vector.tensor_scalar(
                        out_sb[:, col:col + 1], zA, -2.0 * PI, 2.0 * PI,
                        Alu.mult, Alu.add)
                else:
                    colF = RY_COL[fuse + fuse]  # cos/sin of the full angle
                    c_ap = cA[:, colF:colF + 1]
                    s_ap = sA[:, colF:colF + 1]
                    sq2 = sb.tile([128, F // 2], f32, tag=f"sqq{col}")
                    sq2v = sq2[:].rearrange("p (o i) -> p o i", i=1 << b)
                    zQ = zacc[:, 4 * col + 1:4 * col + 2]
                    u_t = zacc[:, 4 * col + 2:4 * col + 3]
                    v1 = zacc[:, 4 * col + 3:4 * col + 4]
                    nc.vector.scalar_tensor_tensor(
                        sq2v, a0, 1.0, a1, Alu.mult, Alu.mult, accum_out=zQ)
                    # u = 2*sin(th)*Q ; v1 = 2*A0 - 1 ; z = cos(th)*v1 - u
                    nc.vector.tensor_scalar(u_t, zQ, s_ap, 2.0,
                                            Alu.mult, Alu.mult)
                    nc.vector.tensor_scalar(v1, zA, 2.0, -1.0,
                                            Alu.mult, Alu.add)
                    nc.vector.scalar_tensor_tensor(
                        v1, v1, c_ap, u_t, Alu.mult, Alu.subtract)
                    # out = pi*(1 - z) = z*(-pi) + pi
                    nc.vector.tensor_scalar(
                        out_sb[:, col:col + 1], v1, -PI, PI,
                        Alu.mult, Alu.add)

            nc.sync.dma_start(out_d, out_sb[:])

    nc.compile()
    return nc


def get_nc():
    if "nc" not in _cache:
        _cache["nc"] = _build_program()
    return _cache["nc"]


def kernel(X, e, Ri, Ro, theta):
    from concourse.bass_utils import run_bass_kernel_spmd

    nc = get_nc()
    in_map = {
        "X": np.ascontiguousarray(np.asarray(X, dtype=np.float32)),
        "e": np.ascontiguousarray(np.asarray(e, dtype=np.float32)),
        "Ri": np.ascontiguousarray(np.asarray(Ri, dtype=np.float32)),
        "Ro": np.ascontiguousarray(np.asarray(Ro, dtype=np.float32)),
        "theta": np.ascontiguousarray(np.asarray(theta, dtype=np.float32)),
    }
    res = run_bass_kernel_spmd(
        nc, [dict(in_map) for _ in range(N_CORES)], core_ids=list(range(N_CORES)),
    )
    return res.results[0]["out"]



# revision 7
# speedup vs baseline: 1.1263x; 1.1263x over previous
"""Trainium2 Bass kernel for nn_NodeNet (GNN message passing + 15-qubit circuit).

Exact algebraic structure exploited (hand-scheduled version):
1. The 2^15 state stays a tensor product of small components; only the
   q5 component reaches 256 dims and only it depends on the message-passing
   matmuls.  The q10 measurement chain depends on X only, so it runs
   entirely inside the input-DMA window.
2. Final CNOT+RY before each measurement are folded into the observable
   (Heisenberg picture): O = cos(th)*Z_c Z_t + sin(th)*X_t, measured with
   3 fused multiply-accumulate ops instead of gate applications.
3. The last CNOT(3,7)+RY(b7) pair on the q5 chain folds into the m5 x m6
   merge by pre-rotating m6 two ways (theta20+-alpha) - the 256-wide RY
   disappears.
4. Range reduction for sin/cos uses mod: sin(u/2) = Sin(pi - 2pi*frac(
   u/(4pi)+16)), one fused ACT op produces both sin and cos columns.
5. DMA plan minimizes HWDGE serialization (a single device in HW): big
   matrices split between the SP HWDGE queue and the Pool SWDGE queue,
   the last-arriving piece is only 128 columns wide.

Self-contained: hardcodes shapes (N=128, E=1024) and the (pre-simplified)
gate structure.
"""

import math

import numpy as np

N_CORES = 8
PI = math.pi

_cache = {}


def _build_program():
    import concourse.bacc as bacc
    import concourse.mybir as mybir
    import concourse.tile as tile
    from concourse.masks import make_identity

    f32 = mybir.dt.float32
    i32 = mybir.dt.int32
    Alu = mybir.AluOpType
    Act = mybir.ActivationFunctionType

    nc = bacc.Bacc(
        "TRN2",
        target_bir_lowering=False,
        debug=False,
        enable_asserts=False,
        num_devices=1,
    )

    X_d = nc.dram_tensor("X", [128, 5], f32, kind="ExternalInput").ap()
    e_d = nc.dram_tensor("e", [1024], f32, kind="ExternalInput").ap()
    Ri_d = nc.dram_tensor("Ri", [128, 1024], f32, kind="ExternalInput").ap()
    Ro_d = nc.dram_tensor("Ro", [128, 1024], f32, kind="ExternalInput").ap()
    th_d = nc.dram_tensor("theta", [31], f32, kind="ExternalInput").ap()
    out_d = nc.dram_tensor("out", [128, 2], f32, kind="ExternalOutput").ap()

    with tile.TileContext(nc) as tc:
        with (
            tc.tile_pool(name="sbuf", bufs=1) as sb,
            tc.tile_pool(name="psmm", bufs=1, space="PSUM") as psmm,
            tc.tile_pool(name="pstp", bufs=1, space="PSUM") as pstp,
        ):
            # ---------------- SBUF tiles ----------------
            Ri_sb = sb.tile([128, 1024], f32, tag="Ri")
            Ro_sb = sb.tile([128, 1024], f32, tag="Ro")
            RiT = sb.tile([128, 1024], f32, tag="RiT")
            RoT = sb.tile([128, 1024], f32, tag="RoT")
            X_sb = sb.tile([128, 5], f32, tag="X")
            TH = sb.tile([128, 31], f32, tag="TH")
            e8_sb = sb.tile([8, 128], f32, tag="e8")
            e_sb = sb.tile([128, 8], f32, tag="e_sb")
            ident = sb.tile([128, 128], f32, tag="ident")

            # ---------------- DMA dispatches ----------------
            # SP HWDGE queue (in order): Ri halves first (earliest big
            # transfers), then theta (broadcast to 128 partitions), e8,
            # and the tail 384 cols of Ro.
            nc.sync.dma_start(Ri_sb[:, 0:512], Ri_d[:, 0:512])
            nc.sync.dma_start(Ri_sb[:, 512:1024], Ri_d[:, 512:1024])
            nc.sync.dma_start(TH[:], th_d.unsqueeze(0).to_broadcast((128, 31)))
            nc.sync.dma_start(e8_sb[:], e_d.rearrange("(c p) -> c p", c=8))
            nc.sync.dma_start(Ro_sb[:, 640:1024], Ro_d[:, 640:1024])

            # Pool SWDGE queue: X, then Ro cols 0-511, then the small
            # 128-col piece (arrives last; short matmul tail).
            nc.gpsimd.dma_start(X_sb[:], X_d)
            make_identity(nc, ident[:])
            nc.gpsimd.dma_start(Ro_sb[:, 0:512], Ro_d[:, 0:512])
            nc.gpsimd.dma_start(Ro_sb[:, 512:640], Ro_d[:, 512:640])

            # ---------------- constants / warmup ----------------
            # Preload the Sin activation table during the DMA window.
            warm = sb.tile([1, 1], f32, tag="warm")
            nc.vector.memset(warm[:], 0.0)
            nc.scalar.activation(warm[:], warm[:], Act.Sin)
            pibias = sb.tile([128, 1], f32, tag="pibias")
            nc.vector.memset(pibias[:], PI)

            # sign row sigma[x] = (-1)^x, replicated across partitions
            sigI = sb.tile([128, 128], i32, tag="sigI")
            sigF = sb.tile([128, 128], f32, tag="sigF")
            nc.gpsimd.iota(sigI[:], pattern=[[1, 128]], base=0,
                           channel_multiplier=0)
            nc.gpsimd.tensor_scalar(sigI[:], sigI[:], 1, None, Alu.bitwise_and)
            nc.gpsimd.tensor_copy(sigF[:], sigI[:])
            nc.gpsimd.tensor_scalar(sigF[:], sigF[:], -2.0, 1.0,
                                    Alu.mult, Alu.add)

            # ---------------- angle block A (theta + X qubits) ----------
            # AANG columns (full angles u; we produce cos/sin of u/2):
            #  0: th14   1: th15   2: th16   3: th19   4: th25
            #  5: th17+th21        6: th24+th27
            #  7: th20+th23+th26   8: th20-th23-th26
            #  9: 2*th29          10: 2*th30
            # 11: X0+th10  12: X1+th11  13: X3+th13+th18+th22
            # 14: X4+th14+th19+th28    15: 2*col14
            AANG = sb.tile([128, 16], f32, tag="AANG")
            scr2 = sb.tile([128, 2], f32, tag="scr2")

            THv = TH[:].rearrange("p (o x) -> p o x", o=1)

            def thcols(lo, n, step=1):
                return THv[:, :, lo:lo + (n - 1) * step + 1:step] if step > 1 \
                    else TH[:, lo:lo + n]

            AAv = AANG[:].rearrange("p (o x) -> p o x", o=1)

            nc.vector.tensor_copy(AANG[:, 0:3], TH[:, 14:17])
            nc.vector.tensor_copy(AAv[:, :, 3:5], thcols(19, 2, 6))
            nc.vector.tensor_tensor(AAv[:, :, 5:7], thcols(17, 2, 7),
                                    thcols(21, 2, 6), Alu.add)
            nc.vector.tensor_tensor(scr2[:, 0:1], TH[:, 23:24], TH[:, 26:27],
                                    Alu.add)
            nc.vector.tensor_tensor(AANG[:, 7:8], TH[:, 20:21], scr2[:, 0:1],
                                    Alu.add)
            nc.vector.tensor_tensor(AANG[:, 8:9], TH[:, 20:21], scr2[:, 0:1],
                                    Alu.subtract)
            nc.vector.tensor_tensor(AANG[:, 9:11], TH[:, 29:31], TH[:, 29:31],
                                    Alu.add)
            nc.vector.tensor_tensor(AANG[:, 11:13], X_sb[:, 0:2], TH[:, 10:12],
                                    Alu.add)
            nc.vector.tensor_tensor(scr2[:], TH[:, 13:15], TH[:, 18:20],
                                    Alu.add)
            nc.vector.tensor_tensor(scr2[:].rearrange("p (o x) -> p o x", o=1),
                                    scr2[:].rearrange("p (o x) -> p o x", o=1),
                                    thcols(22, 2, 6), Alu.add)
            nc.vector.tensor_tensor(AANG[:, 13:15], X_sb[:, 3:5], scr2[:],
                                    Alu.add)
            nc.vector.tensor_tensor(AANG[:, 15:16], AANG[:, 14:15],
                                    AANG[:, 14:15], Alu.add)

            # sincos A: csA[:, j] = sin(u_j/2), csA[:, 16+j] = cos(u_j/2)
            wsA = sb.tile([128, 32], f32, tag="wsA")
            csA = sb.tile([128, 32], f32, tag="csA")
            K4 = 1.0 / (4.0 * PI)
            nc.vector.tensor_scalar(wsA[:, 0:16], AANG[:], K4, 16.0,
                                    Alu.mult, Alu.add)
            nc.vector.tensor_scalar(wsA[:, 16:32], AANG[:], K4, 16.25,
                                    Alu.mult, Alu.add)
            nc.vector.tensor_scalar(wsA[:], wsA[:], 1.0, None, Alu.mod)
            nc.scalar.activation(csA[:], wsA[:], Act.Sin,
                                 bias=pibias[:], scale=-2.0 * PI)

            def sA(j):
                return csA[:, j:j + 1]

            def cA(j):
                return csA[:, 16 + j:16 + j + 1]

            # negated sins for the folded measurements
            nsA = sb.tile([128, 2], f32, tag="nsA")
            nc.vector.tensor_scalar(nsA[:], csA[:, 9:11], -1.0, None, Alu.mult)

            # measurement rows (ready early): Rz = cos(th29)*sigma,
            # Rzneg = -Rz
            Rz = sb.tile([128, 128], f32, tag="Rz")
            Rzn = sb.tile([128, 128], f32, tag="Rzn")
            nc.gpsimd.tensor_scalar(Rz[:], sigF[:], cA(9), None, Alu.mult)
            nc.gpsimd.tensor_scalar(Rzn[:], Rz[:], -1.0, None, Alu.mult)

            # ---------------- q10 chain (X-only, hidden under DMA) -------
            # m4 = merge+cnot(q11(L), q10(H)); RY(b1, col5); m8 = merge+
            # cnot(q13(L), m4(H)); RY(b2, col6); measure folded:
            # z10 = c30*cos(q14_full)*<Z_b2> + 2*s30*<X_b2-pair>
            m4 = sb.tile([128, 4], f32, tag="m4")
            m8 = sb.tile([128, 8], f32, tag="m8")
            d4 = sb.tile([128, 4], f32, tag="d4")
            d8 = sb.tile([128, 8], f32, tag="d8")
            zac = sb.tile([128, 8], f32, tag="zac")
            out_sb = sb.tile([128, 2], f32, tag="out")

            csAv = csA[:].rearrange("p (c d b) -> p c d b", d=2, b=8)
            # L=q11 pair: cols (16+12, 12); H=q10 pair: (16+11, 11)
            m4v = m4[:].rearrange("p (t c) -> p t c", c=2)
            # H[t]: t=0 -> col 27, t=1 -> col 11  (start 27 stride -16)
            Hq10 = csA[:].rearrange("p (c x) -> p c x", c=2)[:, ::-1, 11:12]
            Hq10r = csA[:].rearrange("p (c x) -> p c x", c=2)[:, :, 11:12]
            nc.vector.tensor_tensor(
                m4v[:, :, 0:1], Hq10,
                cA(12).unsqueeze(1).to_broadcast((128, 2, 1)), Alu.mult)
            nc.vector.tensor_tensor(
                m4v[:, :, 1:2], Hq10r,
                sA(12).unsqueeze(1).to_broadcast((128, 2, 1)), Alu.mult)
            # RY(m4, b1, col5): a0=m4[:,0:2] (t=0), a1=m4[:,2:4]
            nc.vector.tensor_scalar(d4[:], m4[:], sA(5), None, Alu.mult)
            nc.vector.scalar_tensor_tensor(
                m4[:, 0:2], m4[:, 0:2], cA(5), d4[:, 2:4],
                Alu.mult, Alu.subtract)
            nc.vector.scalar_tensor_tensor(
                m4[:, 2:4], m4[:, 2:4], cA(5), d4[:, 0:2],
                Alu.mult, Alu.add)
            # m8 = merge+cnot(q13(L,b0), m4(H,b1-b2)); ctrl b0, tgt b2
            m8v = m8[:].rearrange("p (h c) -> p h c", c=2)
            m4f = m4[:].rearrange("p (t i) -> p t i", t=2)
            nc.vector.tensor_tensor(
                m8v[:, :, 0:1], m4[:].unsqueeze(2),
                cA(13).unsqueeze(1).to_broadcast((128, 4, 1)), Alu.mult)
            m8v2 = m8[:].rearrange("p (hb1 hb0 c) -> p hb1 hb0 c",
                                   hb0=2, c=2)
            nc.vector.tensor_tensor(
                m8v2[:, :, :, 1:2],
                m4f[:, ::-1, :].unsqueeze(3),
                sA(13).unsqueeze(1).unsqueeze(1)
                .to_broadcast((128, 2, 2, 1)), Alu.mult)
            # RY(m8, b2, col6): a0 = cols 0-3, a1 = cols 4-7
            nc.vector.tensor_scalar(d8[:], m8[:], sA(6), None, Alu.mult)
            nc.vector.scalar_tensor_tensor(
                m8[:, 0:4], m8[:, 0:4], cA(6), d8[:, 4:8],
                Alu.mult, Alu.subtract)
            nc.vector.scalar_tensor_tensor(
                m8[:, 4:8], m8[:, 4:8], cA(6), d8[:, 0:4],
                Alu.mult, Alu.add)
            # measure: w0=sum(a0^2), w1=sum(a1^2), w2=sum(a0*a1)
            nc.vector.scalar_tensor_tensor(d8[:, 0:4], m8[:, 0:4], 1.0,
                                           m8[:, 0:4], Alu.mult, Alu.mult,
                                           accum_out=zac[:, 0:1])
            nc.vector.scalar_tensor_tensor(d8[:, 4:8], m8[:, 4:8], 1.0,
                                           m8[:, 4:8], Alu.mult, Alu.mult,
                                           accum_out=zac[:, 1:2])
            nc.vector.scalar_tensor_tensor(d4[:, 0:4], m8[:, 0:4], 1.0,
                                           m8[:, 4:8], Alu.mult, Alu.mult,
                                           accum_out=zac[:, 2:3])
            # z10 = cA(10)*cA(15)*(w0-w1) - 2*sA(10)*w2
            nc.vector.tensor_tensor(zac[:, 3:4], zac[:, 0:1], zac[:, 1:2],
                                    Alu.subtract)
            nc.vector.tensor_scalar(zac[:, 3:4], zac[:, 3:4], cA(15), cA(10),
                                    Alu.mult, Alu.mult)
            nc.vector.tensor_tensor(zac[:, 4:5], zac[:, 2:3], zac[:, 2:3],
                                    Alu.add)
            nc.vector.scalar_tensor_tensor(
                zac[:, 4:5], zac[:, 4:5], nsA[:, 1:2], zac[:, 3:4],
                Alu.mult, Alu.add)
            nc.vector.tensor_scalar(out_sb[:, 1:2], zac[:, 4:5], -PI, PI,
                                    Alu.mult, Alu.add)

            # ---------------- message passing pipeline ----------------
            # e transpose: [8,128] -> [128,8]
            e_ps = psmm.tile([128, 8], f32, tag="e_ps")
            nc.tensor.transpose(e_ps[:], e8_sb[:], ident[0:8, 0:8])
            nc.scalar.copy(e_sb[:], e_ps[:])

            # bo/bi: bb_ps[:, c*10:+5] = Ro_c^T X ; +5:+10 = Ri_c^T X
            bb_ps = psmm.tile([128, 80], f32, tag="bb")
            for c in range(8):
                nc.tensor.matmul(bb_ps[:, c * 10 + 5:c * 10 + 10],
                                 Ri_sb[:, c * 128:(c + 1) * 128], X_sb[:],
                                 start=True, stop=True)
            for c in range(8):
                nc.tensor.matmul(bb_ps[:, c * 10:c * 10 + 5],
                                 Ro_sb[:, c * 128:(c + 1) * 128], X_sb[:],
                                 start=True, stop=True)

            # transposes: Ri chunks then Ro chunks; copies alternate
            # DVE / ACT / Pool.
            def copy_out(dst, src, eng):
                if eng == 0:
                    nc.vector.tensor_copy(dst, src)
                elif eng == 1:
                    nc.scalar.copy(dst, src)
                else:
                    nc.gpsimd.tensor_copy(dst, src)

            for h in range(2):
                tp = pstp.tile([128, 512], f32, tag=f"tpri{h}")
                for cc in range(4):
                    c = h * 4 + cc
                    nc.tensor.transpose(tp[:, cc * 128:(cc + 1) * 128],
                                        Ri_sb[:, c * 128:(c + 1) * 128],
                                        ident[:])
                copy_out(RiT[:, h * 512:h * 512 + 256], tp[:, 0:256], h)
                copy_out(RiT[:, h * 512 + 256:h * 512 + 512], tp[:, 256:512],
                         1 - h)
            for h in range(2):
                tp = pstp.tile([128, 512], f32, tag=f"tpro{h}")
                for cc in range(4):
                    c = h * 4 + cc
                    nc.tensor.transpose(tp[:, cc * 128:(cc + 1) * 128],
                                        Ro_sb[:, c * 128:(c + 1) * 128],
                                        ident[:])
                if h == 0:
                    copy_out(RoT[:, 0:256], tp[:, 0:256], 0)
                    copy_out(RoT[:, 256:512], tp[:, 256:512], 1)
                else:
                    # last piece 512:640 arrives latest - copy its
                    # transpose separately on DVE for the shortest tail
                    copy_out(RoT[:, 640:1024], tp[:, 128:512], 1)
                    copy_out(RoT[:, 512:640], tp[:, 0:128], 0)

            # weight by e: bow[:, c*10:+10] = bb[:, ...] * e_c
            bow = sb.tile([128, 80], f32, tag="bow")
            for g in range(2):
                ev = e_sb[:, g * 4:(g + 1) * 4].rearrange(
                    "p (c o) -> p c o", o=1).to_broadcast((128, 4, 10))
                ov = bow[:, g * 40:(g + 1) * 40].rearrange(
                    "p (c j) -> p c j", j=10)
                iv = bb_ps[:, g * 40:(g + 1) * 40].rearrange(
                    "p (c j) -> p c j", j=10)
                nc.vector.tensor_tensor(ov, iv, ev, Alu.mult)

            # mi/mo accumulation: mm_ps cols 0-4 = mi, 5-9 = mo
            mm_ps = psmm.tile([128, 10], f32, tag="mm")
            for c in range(8):
                nc.tensor.matmul(mm_ps[:, 0:5],
                                 RiT[:, c * 128:(c + 1) * 128],
                                 bow[:, c * 10:c * 10 + 5],
                                 start=(c == 0), stop=(c == 7))
                nc.tensor.matmul(mm_ps[:, 5:10],
                                 RoT[:, c * 128:(c + 1) * 128],
                                 bow[:, c * 10 + 5:c * 10 + 10],
                                 start=(c == 0), stop=(c == 7))

            # ---------------- sincos B (critical path) ----------------
            # angle_q = mm_ps[:, q] + theta_q  (q = 0..7)
            rowB = sb.tile([128, 8], f32, tag="rowB")
            rowB2 = sb.tile([128, 8], f32, tag="rowB2")
            nc.vector.tensor_scalar(rowB[:], TH[:, 0:8], K4, 16.0,
                                    Alu.mult, Alu.add)
            nc.vector.tensor_scalar(rowB2[:], TH[:, 0:8], K4, 16.25,
                                    Alu.mult, Alu.add)

            wsB = sb.tile([128, 16], f32, tag="wsB")
            csB = sb.tile([128, 16], f32, tag="csB")
            nc.vector.scalar_tensor_tensor(wsB[:, 0:8], mm_ps[:, 0:8], K4,
                                           rowB[:], Alu.mult, Alu.add)
            nc.gpsimd.scalar_tensor_tensor(wsB[:, 8:16], mm_ps[:, 0:8], K4,
                                           rowB2[:], Alu.mult, Alu.add)
            nc.vector.tensor_scalar(wsB[:, 0:8], wsB[:, 0:8], 1.0, None,
                                    Alu.mod)
            nc.gpsimd.tensor_scalar(wsB[:, 8:16], wsB[:, 8:16], 1.0, None,
                                    Alu.mod)
            nc.scalar.activation(csB[:], wsB[:], Act.Sin,
                                 bias=pibias[:], scale=-2.0 * PI)
            # csB: sin(q) at col q, cos(q) at col 8+q

            # ---------------- q5 evolution ----------------
            # Level 0 (batched): mA = {m0=(q0,q1), m2=(q4,q5)},
            #                    mB = {m1=(q3,q2), m3=(q7,q6)}
            # layout: col = comp + 2*b0 + 4*b1   (b0 = L bit, b1 = H bit)
            mA = sb.tile([128, 8], f32, tag="mA")
            mB = sb.tile([128, 8], f32, tag="mB")
            csBv = csB[:].rearrange("p (c d b) -> p c d b", d=2, b=4)

            def level0(mt, lq, hq, eng):
                # L pair cols (8+lq, lq); H pair (8+hq, hq); comps lq,lq+4
                tt = nc.vector.tensor_tensor if eng == 0 else \
                    nc.gpsimd.tensor_tensor
                ov = mt[:].rearrange("p (b1 b0 c) -> p b1 b0 c", b0=2, c=2)
                # operand views: col(H) = 8 - 8*b1 + 4*comp + hq
                Hb = csB[:].rearrange("p (c d b) -> p c d b", d=2, b=4)[
                    :, ::-1, :, hq:hq + 1].rearrange("p c d o -> p c (d o)")
                Hbr = csB[:].rearrange("p (c d b) -> p c d b", d=2, b=4)[
                    :, :, :, hq:hq + 1].rearrange("p c d o -> p c (d o)")
                Lcb = csBv[:, 1, :, lq:lq + 1].rearrange("p d o -> p (d o)")\
                    .unsqueeze(1).to_broadcast((128, 2, 2))
                Lsb = csBv[:, 0, :, lq:lq + 1].rearrange("p d o -> p (d o)")\
                    .unsqueeze(1).to_broadcast((128, 2, 2))
                tt(ov[:, :, 0, :], Lcb, Hb, Alu.mult)
                tt(ov[:, :, 1, :], Lsb, Hbr, Alu.mult)

            level0(mA, 0, 1, 0)   # m0=(q0 ctrl, q1 tgt), m2=(q4, q5) on DVE
            level0(mB, 3, 2, 1)   # m1=(q3, q2), m3=(q7, q6) on Pool

            # b1 RYs: mA comps (m0: th15, m2: th14) -> csA cols (1, 0)
            #         mB comps (m1: th16, m3: th15) -> csA cols (2, 1)
            sc4 = sb.tile([128, 4], f32, tag="sc4")
            sc4b = sb.tile([128, 4], f32, tag="sc4b")
            sc4c = sb.tile([128, 4], f32, tag="sc4c")
            sc4d = sb.tile([128, 4], f32, tag="sc4d")
            dA = sb.tile([128, 8], f32, tag="dA")
            dB = sb.tile([128, 8], f32, tag="dB")

            def ry_b1_batch(mt, c_hi, scv, scv2, dt, eng):
                # coefs: comp0 at csA col c_hi, comp1 at col c_hi-1
                tt = nc.vector.tensor_tensor if eng == 0 else \
                    nc.gpsimd.tensor_tensor
                cview = csA[:].rearrange("p (o x) -> p o x", o=1)[
                    :, :, 16 + c_hi - 1:16 + c_hi + 1][:, :, ::-1]
                sview = csA[:].rearrange("p (o x) -> p o x", o=1)[
                    :, :, c_hi - 1:c_hi + 1][:, :, ::-1]
                cb = cview.to_broadcast((128, 2, 2))
                sb_ = sview.unsqueeze(1).to_broadcast((128, 2, 2, 2))
                a0 = mt[:, 0:4].rearrange("p (b0 c) -> p b0 c", c=2)
                a1 = mt[:, 4:8].rearrange("p (b0 c) -> p b0 c", c=2)
                dv = dt[:].rearrange("p (b1 b0 c) -> p b1 b0 c", b0=2, c=2)
                t0 = scv[:].rearrange("p (b0 c) -> p b0 c", c=2)
                t1 = scv2[:].rearrange("p (b0 c) -> p b0 c", c=2)
                tt(t0, a0, cb, Alu.mult)
                tt(t1, a1, cb, Alu.mult)
                tt(dv, mt[:].rearrange("p (b1 b0 c) -> p b1 b0 c", b0=2, c=2),
                   sb_, Alu.mult)
                tt(a0, t0, dv[:, 1], Alu.subtract)
                tt(a1, t1, dv[:, 0], Alu.add)

            ry_b1_batch(mA, 1, sc4, sc4b, dA, 0)
            ry_b1_batch(mB, 2, sc4c, sc4d, dB, 1)

            # b0 RY on m0 (th25 = csA col 4): m0 = mA comp 0, strided
            m0v = mA[:].rearrange("p (b1 b0 c) -> p b1 b0 c", b0=2, c=2)
            nc.vector.tensor_scalar(
                dA[:].rearrange("p (b1 b0 c) -> p b1 b0 c", b0=2, c=2)
                [:, :, :, 0:1],
                m0v[:, :, :, 0:1], sA(4), None, Alu.mult)
            dAv = dA[:].rearrange("p (b1 b0 c) -> p b1 b0 c", b0=2, c=2)
            nc.vector.scalar_tensor_tensor(
                m0v[:, :, 0, 0:1], m0v[:, :, 0, 0:1], cA(4),
                dAv[:, :, 1, 0:1], Alu.mult, Alu.subtract)
            nc.vector.scalar_tensor_tensor(
                m0v[:, :, 1, 0:1], m0v[:, :, 1, 0:1], cA(4),
                dAv[:, :, 0, 0:1], Alu.mult, Alu.add)

            # m5 = merge+cnot(m0, m1; ctrl=b1 of m0, tgt=b1 of m1) on DVE
            # m6 = merge+cnot(m3, m2; same) on Pool
            # m56: m5 = cols 0-15, m6 = 16-31; col = l + 4*h
            m56 = sb.tile([128, 32], f32, tag="m56")

            def merge_cnot_l1h3(dst_off, Lt, l_comp, Ht, h_comp, eng):
                tt = nc.vector.tensor_tensor if eng == 0 else \
                    nc.gpsimd.tensor_tensor
                # L[cb, v0] at col l_comp + 2*v0 + 4*cb
                Lv = Lt[:].rearrange("p (cb v0 c) -> p cb v0 c", v0=2, c=2)
                Hv = Ht[:].rearrange("p (hb1 hb0 c) -> p hb1 hb0 c",
                                     hb0=2, c=2)
                ov = m56[:, dst_off:dst_off + 16].rearrange(
                    "p (hb1 hb0 cb v0) -> p hb1 hb0 cb v0", hb0=2, cb=2, v0=2)
                # cb=0: out = L[0, v0] * H[hb1, hb0]
                tt(ov[:, :, :, 0, :],
                   Lv[:, 0, :, l_comp:l_comp + 1].rearrange("p v o -> p (v o)")
                   .unsqueeze(1).unsqueeze(1).to_broadcast((128, 2, 2, 2)),
                   Hv[:, :, :, h_comp:h_comp + 1]
                   .to_broadcast((128, 2, 2, 2)),
                   Alu.mult)
                # cb=1: out = L[1, v0] * H[1-hb1, hb0]
                tt(ov[:, :, :, 1, :],
                   Lv[:, 1, :, l_comp:l_comp + 1].rearrange("p v o -> p (v o)")
                   .unsqueeze(1).unsqueeze(1).to_broadcast((128, 2, 2, 2)),
                   Hv[:, ::-1, :, h_comp:h_comp + 1]
                   .to_broadcast((128, 2, 2, 2)),
                   Alu.mult)

            merge_cnot_l1h3(0, mA, 0, mB, 0, 0)    # m5 on DVE
            merge_cnot_l1h3(16, mB, 1, mA, 1, 1)   # m6 on Pool

            # RY(m5, b3, th19 = csA col 3) on DVE
            d16 = sb.tile([128, 16], f32, tag="d16")
            nc.vector.tensor_scalar(d16[:], m56[:, 0:16], sA(3), None,
                                    Alu.mult)
            nc.vector.scalar_tensor_tensor(
                m56[:, 0:8], m56[:, 0:8], cA(3), d16[:, 8:16],
                Alu.mult, Alu.subtract)
            nc.vector.scalar_tensor_tensor(
                m56[:, 8:16], m56[:, 8:16], cA(3), d16[:, 0:8],
                Alu.mult, Alu.add)

            # H0 = RY(th20+a)(m6), G = RY(th20-a)(m6)  [a = th23+th26]
            # csA col 7 = (th20+a), col 8 = (th20-a); on Pool
            h0t = sb.tile([128, 16], f32, tag="h0t")
            gt = sb.tile([128, 16], f32, tag="gt")
            da = sb.tile([128, 16], f32, tag="da")
            db = sb.tile([128, 16], f32, tag="db")
            da2 = sb.tile([128, 16], f32, tag="da2")
            db2 = sb.tile([128, 16], f32, tag="db2")
            m6v = m56[:, 16:32]
            nc.gpsimd.tensor_scalar(da[:], m6v, sA(7), None, Alu.mult)
            nc.gpsimd.tensor_scalar(db[:], m6v, cA(7), None, Alu.mult)
            nc.gpsimd.tensor_tensor(h0t[:, 0:8], db[:, 0:8], da[:, 8:16],
                                    Alu.subtract)
            nc.gpsimd.tensor_tensor(h0t[:, 8:16], da[:, 0:8], db[:, 8:16],
                                    Alu.add)
            nc.gpsimd.tensor_scalar(da2[:], m6v, sA(8), None, Alu.mult)
            nc.gpsimd.tensor_scalar(db2[:], m6v, cA(8), None, Alu.mult)
            nc.gpsimd.tensor_tensor(gt[:, 0:8], db2[:, 0:8], da2[:, 8:16],
                                    Alu.subtract)
            nc.gpsimd.tensor_tensor(gt[:, 8:16], da2[:, 0:8], db2[:, 8:16],
                                    Alu.add)

            # m7 = merge: cols l + 16*h; cb = m5 b3 (l in 8-15)
            m7 = sb.tile([128, 256], f32, tag="m7")
            m7v = m7[:].rearrange("p (h l) -> p h l", l=16)
            nc.vector.tensor_tensor(
                m7v[:, :, 0:8],
                m56[:, 0:8].unsqueeze(1).to_broadcast((128, 16, 8)),
                h0t[:].unsqueeze(2).to_broadcast((128, 16, 8)),
                Alu.mult)
            m7v2 = m7[:].rearrange("p (hb hl l) -> p hb hl l", hl=8, l=16)
            nc.gpsimd.tensor_tensor(
                m7v2[:, :, :, 8:16],
                m56[:, 8:16].unsqueeze(1).unsqueeze(1)
                .to_broadcast((128, 2, 8, 8)),
                gt[:].rearrange("p (b x) -> p b x", b=2)[:, ::-1, :]
                .unsqueeze(3).to_broadcast((128, 2, 8, 8)),
                Alu.mult)

            # measurement: z = sum a0*(Rz a0 + s29 a1) + sum a1*(Rzn a1
            # + s29 a0);  a0 = m7[:, 0:128], a1 = m7[:, 128:256]
            w0 = sb.tile([128, 128], f32, tag="w0")
            w1 = sb.tile([128, 128], f32, tag="w1")
            a0 = m7[:, 0:128]
            a1 = m7[:, 128:256]
            nc.vector.tensor_tensor(w0[:], a0, Rz[:], Alu.mult)
            nc.vector.scalar_tensor_tensor(w0[:], a1, nsA[:, 0:1], w0[:],
                                           Alu.mult, Alu.add)
            nc.vector.scalar_tensor_tensor(w0[:], a0, 1.0, w0[:],
                                           Alu.mult, Alu.mult,
                                           accum_out=zac[:, 5:6])
            nc.gpsimd.tensor_tensor(w1[:], a1, Rzn[:], Alu.mult)
            nc.gpsimd.scalar_tensor_tensor(w1[:], a0, nsA[:, 0:1], w1[:],
                                           Alu.mult, Alu.add)
            nc.gpsimd.scalar_tensor_tensor(w1[:], a1, 1.0, w1[:],
                                           Alu.mult, Alu.mult,
                                           accum_out=zac[:, 6:7])
            nc.vector.tensor_tensor(zac[:, 7:8], zac[:, 5:6], zac[:, 6:7],
                                    Alu.add)
            nc.vector.tensor_scalar(out_sb[:, 0:1], zac[:, 7:8], -PI, PI,
                                    Alu.mult, Alu.add)

            nc.sync.dma_start(out_d, out_sb[:])

    nc.compile()
    return nc


def get_nc():
    if "nc" not in _cache:
        _cache["nc"] = _build_program()
    return _cache["nc"]


def kernel(X, e, Ri, Ro, theta):
    from concourse.bass_utils import run_bass_kernel_spmd

    nc = get_nc()
    in_map = {
        "X": np.ascontiguousarray(np.asarray(X, dtype=np.float32)),
        "e": np.ascontiguousarray(np.asarray(e, dtype=np.float32)),
        "Ri": np.ascontiguousarray(np.asarray(Ri, dtype=np.float32)),
        "Ro": np.ascontiguousarray(np.asarray(Ro, dtype=np.float32)),
        "theta": np.ascontiguousarray(np.asarray(theta, dtype=np.float32)),
    }
    res = run_bass_kernel_spmd(
        nc, [dict(in_map) for _ in range(N_CORES)],
        core_ids=list(range(N_CORES)),
    )
    return res.results[0]["out"]
